# revision 51
# baseline (speedup 1.0000x reference)
import sys
from contextlib import ExitStack

import numpy as np
import ml_dtypes

sys.path.insert(0, "/opt/trn_rl_repo")

import concourse.bass as bass
import concourse.tile as tile
from concourse import bacc, mybir
from concourse.bass_utils import run_bass_kernel_spmd

B, H, W, CH = 4, 80, 80, 256
NCLS, DIM = 22, 256
ROWS = 40            # rows per core
NPIX = ROWS * W      # 3200 output pixels per core
NT = (ROWS + 2) * W + 2   # 3362 strip positions (1 halo row each side + 1 elem pad)
NTILE = NPIX // 128  # 25 output tiles of 128 pixels
F32 = mybir.dt.float32
BF16 = mybir.dt.bfloat16
BF16NP = ml_dtypes.bfloat16

# pixel tiles are processed in groups; conv weights stay stationary on the PE
# across a whole group (N=G*128 moving columns per LDWEIGHTS).
# patch taps are ordered kk = j*3 + i (j = column offset, i = row offset) so
# each gather op (fixed j) reads/writes contiguous kk blocks.
GROUPS = [4, 8, 8, 5]          # tiles per group (sum = 25)
G_T0 = [0, 4, 12, 20]          # first tile of each group
KK_ORDER = [0, 1, 2, 6, 7, 8, 3, 4, 5]   # j=0, j=2, j=1 (xs needed last)
WARMUP = 0                     # HAM warmup matmuls before group 0 (0 = off)
SEL_BASE = [t0 * 9 * 128 for t0 in G_T0]   # selt column base per group


def _ap(t, off, dims):
    # raw AP on a pool tile's backing tensor: partition dim + free dims
    base = t[:, 0:1]
    return bass.AP(base.tensor, base.offset + off,
                   [[base.ap[0][0], 128]] + dims)


def _build_nc():
    nc = bacc.Bacc("TRN2", target_bir_lowering=False, debug=False,
                   enable_asserts=False, num_devices=8)
    xt_d = nc.dram_tensor("xt", [128, 2 * NT], BF16, kind="ExternalInput").ap()
    wt_d = nc.dram_tensor("wt", [128, 36 * 128], BF16, kind="ExternalInput").ap()
    selt_d = nc.dram_tensor("selt", [128, NTILE * 9 * 128], BF16,
                            kind="ExternalInput").ap()
    out_d = nc.dram_tensor("out", [2 * 128, NPIX], BF16,
                           kind="ExternalOutput").ap()

    with tile.TileContext(nc) as tc, ExitStack() as ctx:
        xp = ctx.enter_context(tc.tile_pool(name="xp", bufs=1))
        wp = ctx.enter_context(tc.tile_pool(name="wp", bufs=1))
        sbp = ctx.enter_context(tc.tile_pool(name="sbp", bufs=1))
        xtsp = ctx.enter_context(tc.tile_pool(name="xtsp", bufs=3))
        outp = ctx.enter_context(tc.tile_pool(name="outp", bufs=2))
        zp = ctx.enter_context(tc.tile_pool(name="zp", bufs=2, space="PSUM"))

        xt = xp.tile([128, 2 * NT], BF16)
        wt = wp.tile([128, 36 * 128], BF16)
        S = sbp.tile([128, NTILE * 9 * 128], BF16)

        # --- input DMAs, ordered so group 0 can start ASAP ---
        # sel is pre-broadcast on the host. Group 0's sel rides the sync
        # queue head alongside wt/xt so both queues feed the startup; the
        # scalar queue streams the later groups' sel from t=0.
        # per-(group, j) chunks: fine-grained completion feeds the gather
        # pipeline incrementally (coarser chunks stall the PE at boundaries)
        def sel_dma(eng, gi, js=(0, 2, 1)):
            G = GROUPS[gi]
            for j in js:          # match gather consumption order
                b0, cols = SEL_BASE[gi] + j * 3 * G * 128, 3 * G * 128
                eng.dma_start(S[:, b0:b0 + cols], selt_d[:, b0:b0 + cols])

        for gi in range(len(GROUPS)):
            sel_dma(nc.scalar, gi)

        bnds = [0, 810, NT]
        nc.sync.dma_start(wt[:, 0:512], wt_d[:, 0:512])

        def xt_dma(a, b):
            # both channel-halves in one DMA so they arrive together
            xb = xt[:, 0:1]
            dst = bass.AP(xb.tensor, xb.offset + a,
                          [[xb.ap[0][0], 128], [NT, 2], [1, b - a]])
            src = bass.AP(xt_d.tensor, a, [[2 * NT, 128], [NT, 2], [1, b - a]])
            nc.sync.dma_start(dst, src)

        xt_dma(bnds[0], bnds[1])
        nc.sync.dma_start(wt[:, 512:], wt_d[:, 512:])
        xt_dma(bnds[1], bnds[2])

        warm = wp.tile([128, 128], BF16)
        nc.gpsimd.memset(warm[:], 0)


        for gi, G in enumerate(GROUPS):
            t0 = G_T0[gi]
            gw = G * 128          # moving columns in this group
            # gated patches, per h: xts_h[c, kk*gw + t*128 + p]
            #   = xt[c, h*NT + (t0+t)*128 + i*80 + j + p] * sel[kk, pixel]
            # one op per (h, j): out and sel are contiguous, only the xt
            # gather is strided; even j is 4B-aligned -> DVE 2x mode.
            # gpsimd (mode-agnostic) takes the misaligned j=1 ops.
            xts = [xtsp.tile([128, 9 * gw], BF16, name=f"xts{h}")
                   for h in range(2)]
            for j in (0, 2, 1):
                for h in range(2):
                    # all on vector: DVE tensor_tensor and any gpsimd op
                    # fight for the same shared SBUF port (exclusive lock),
                    # so splitting across engines only adds blocking.
                    # j=1 is 2B-misaligned -> 1x mode; still fits under PE.
                    o = _ap(xts[h], j * 3 * gw, [[1, 3 * gw]])
                    i1 = _ap(xt, h * NT + t0 * 128 + j,
                             [[80, 3], [128, G], [1, 128]])
                    i2 = _ap(S, SEL_BASE[gi] + j * 3 * gw, [[1, 3 * gw]])
                    nc.vector.tensor_mul(o, i1, i2)

            # PE: stationary = w[kk,h,dh] chunk, moving = gated patches.
            # z[d, dh*1024 + p], accumulated over (kk, h) in PSUM.
            z = zp.tile([128, 2048], F32)
            if gi == 0 and WARMUP:
                # warm up the PE's HAM clock gate while input DMAs ramp;
                # overwritten by the first real accumulation (start=True)
                for _ in range(WARMUP):
                    nc.tensor.matmul(z[:, 0:64], warm[:], warm[:, 0:64],
                                     start=True, stop=True,
                                     skip_group_check=True)
            outt = outp.tile([128, 2 * gw], BF16)

            def mm(kk, h, dh, ki):
                wc = wt[:, ((kk * 2 + h) * 2 + dh) * 128:
                        ((kk * 2 + h) * 2 + dh + 1) * 128]
                for n0 in range(0, gw, 512):
                    n1 = min(n0 + 512, gw)
                    nc.tensor.matmul(
                        z[:, dh * 1024 + n0:dh * 1024 + n1],
                        wc,
                        xts[h][:, kk * gw + n0:kk * gw + n1],
                        start=(ki == 0 and h == 0),
                        stop=(ki == 8 and h == 1))

            def evac(dh):
                # PSUM -> SBUF bf16, then contiguous DMA to DRAM [d, p]
                nc.scalar.copy(outt[:, dh * gw:(dh + 1) * gw],
                               z[:, dh * 1024:dh * 1024 + gw])
                nc.sync.dma_start(
                    out_d[dh * 128:(dh + 1) * 128, t0 * 128:t0 * 128 + gw],
                    outt[:, dh * gw:(dh + 1) * gw])

            for ki, kk in enumerate(KK_ORDER):
                for h in range(2):
                    for dh in range(2):
                        mm(kk, h, dh, ki)
            evac(0)
            evac(1)
    nc.compile()
    return nc


_NC_CACHE = None


def _get_nc():
    global _NC_CACHE
    if _NC_CACHE is None:
        _NC_CACHE = _build_nc()
    return _NC_CACHE


def _prep_core(x, seg_mask, core):
    b, r0 = core // 2, 40 * (core % 2)
    xp = np.pad(x[b], ((1, 1), (0, 0), (0, 0)))        # [82,80,256]
    strip = xp[r0:r0 + 42].reshape(42 * W, CH)
    sp = np.zeros((NT, CH), np.float32)
    sp[1:1 + 42 * W] = strip
    spT = sp.T
    xt = np.ascontiguousarray(
        np.concatenate([spT[:128], spT[128:]], axis=1)).astype(BF16NP)

    pads = np.pad(seg_mask[b], ((1, 1), (1, 1), (0, 0)))  # [82,82,22]
    mc = seg_mask[b][r0:r0 + 40]                          # [40,80,22]
    smax = mc.max(-1, keepdims=True)
    eq = (mc == smax).astype(np.float32)
    sel = np.empty((40, 80, 9), np.float32)
    for k in range(9):
        di, dj = k // 3 - 1, k % 3 - 1
        sel[..., k] = (eq * pads[r0 + 1 + di:r0 + 41 + di,
                                 1 + dj:81 + dj]).sum(-1)
    cnt = (sel != 0).astype(np.float32).sum(-1, keepdims=True)
    selp = (sel * (9.0 / np.maximum(cnt, 1.0))).reshape(NPIX, 9)
    # group-major, kk = j*3+i ordered: col = SEL_BASE[g] + kk*(G*128) + t*128 + p
    KK2K = [(kk % 3) * 3 + kk // 3 for kk in range(9)]   # kk -> k = i*3+j
    parts = []
    for gi, G in enumerate(GROUPS):
        t0 = G_T0[gi]
        blk = selp[t0 * 128:(t0 + G) * 128][:, KK2K]     # [G*128, kk]
        parts.append(blk.reshape(G, 128, 9).transpose(2, 0, 1).reshape(-1))
    selt = np.concatenate(parts).astype(BF16NP).reshape(1, NTILE * 9 * 128)
    # pre-broadcast across the 128 partitions (plain contiguous DMA on-chip)
    selt = np.ascontiguousarray(np.broadcast_to(selt, (128, NTILE * 9 * 128)))
    return xt, selt


def _build_in_maps(x, seg_mask, conv_w):
    w9 = conv_w.reshape(2, 128, 9, 2, 128)               # [h, c, k, dh, d]
    KK2K = [(kk % 3) * 3 + kk // 3 for kk in range(9)]
    # wt[c, ((kk*2+h)*2+dh)*128 + d] = w[h*128+c, KK2K[kk], dh*128+d]
    wt = np.ascontiguousarray(
        w9[:, :, KK2K]                                   # [h, c, kk, dh, d]
          .transpose(1, 2, 0, 3, 4)                      # [c, kk, h, dh, d]
          .reshape(128, 36 * 128)).astype(BF16NP)

    in_maps = []
    for core in range(8):
        xt, selt = _prep_core(x, seg_mask, core)
        in_maps.append({"xt": xt, "wt": wt, "selt": selt})
    return in_maps


def kernel(x, seg_mask, conv_w):
    x = np.asarray(x, np.float32)
    seg_mask = np.asarray(seg_mask, np.float32)
    conv_w = np.asarray(conv_w, np.float32)

    in_maps = _build_in_maps(x, seg_mask, conv_w)
    nc = _get_nc()
    res = run_bass_kernel_spmd(nc, in_maps, core_ids=list(range(8)))

    out = np.empty((B, H, W, DIM), np.float32)
    for core in range(8):
        b, r0 = core // 2, 40 * (core % 2)
        o = res.results[core]["out"].astype(np.float32)   # [256, 3200]
        out[b, r0:r0 + 40] = o.T.reshape(ROWS, W, DIM)
    return out


# revision 52
# speedup vs baseline: 1.1180x; 1.1180x over previous
import sys
from contextlib import ExitStack

import numpy as np
import ml_dtypes

sys.path.insert(0, "/opt/trn_rl_repo")

import concourse.bass as bass
import concourse.tile as tile
from concourse import bacc, mybir
from concourse.bass_utils import run_bass_kernel_spmd

B, H, W, CH = 4, 80, 80, 256
NCLS, DIM = 22, 256
ROWS = 40            # rows per core
NPIX = ROWS * W      # 3200 output pixels per core
NT = (ROWS + 2) * W + 2   # 3362 strip positions (1 halo row each side + 1 elem pad)
NTILE = NPIX // 128  # 25 output tiles of 128 pixels
F32 = mybir.dt.float32
BF16 = mybir.dt.bfloat16
BF16NP = ml_dtypes.bfloat16

# pixel tiles are processed in groups; conv weights stay stationary on the PE
# across a whole group (N=G*128 moving columns per LDWEIGHTS).
# patch taps are ordered kk = j*3 + i (j = column offset, i = row offset) so
# each gather op (fixed j) reads/writes contiguous kk blocks.
GROUPS = [4, 8, 8, 5]          # tiles per group (sum = 25)
G_T0 = [0, 4, 12, 20]          # first tile of each group
KK_ORDER = [0, 1, 2, 6, 7, 8, 3, 4, 5]   # j=0, j=2, j=1 (xs needed last)
WARMUP = 0                     # HAM warmup matmuls before group 0 (0 = off)
SEL_BASE = [t0 * 9 * 128 for t0 in G_T0]   # selt column base per group


def _ap(t, off, dims):
    # raw AP on a pool tile's backing tensor: partition dim + free dims
    base = t[:, 0:1]
    return bass.AP(base.tensor, base.offset + off,
                   [[base.ap[0][0], 128]] + dims)


def _build_nc():
    nc = bacc.Bacc("TRN2", target_bir_lowering=False, debug=False,
                   enable_asserts=False, num_devices=8)
    xt_d = nc.dram_tensor("xt", [128, 2 * NT], BF16, kind="ExternalInput").ap()
    wt_d = nc.dram_tensor("wt", [128, 36 * 128], BF16, kind="ExternalInput").ap()
    selt_d = nc.dram_tensor("selt", [128, NTILE * 9 * 128], BF16,
                            kind="ExternalInput").ap()
    out_d = nc.dram_tensor("out", [2 * 128, NPIX], BF16,
                           kind="ExternalOutput").ap()

    with tile.TileContext(nc) as tc, ExitStack() as ctx:
        xp = ctx.enter_context(tc.tile_pool(name="xp", bufs=1))
        wp = ctx.enter_context(tc.tile_pool(name="wp", bufs=1))
        sbp = ctx.enter_context(tc.tile_pool(name="sbp", bufs=1))
        xtsp = ctx.enter_context(tc.tile_pool(name="xtsp", bufs=3))
        outp = ctx.enter_context(tc.tile_pool(name="outp", bufs=2))
        zp = ctx.enter_context(tc.tile_pool(name="zp", bufs=2, space="PSUM"))

        xt = xp.tile([128, 2 * NT], BF16)
        wt = wp.tile([128, 36 * 128], BF16)
        S = sbp.tile([128, NTILE * 9 * 128], BF16)

        # --- input DMAs, ordered so group 0 can start ASAP ---
        # sel is pre-broadcast on the host. Group 0's sel rides the sync
        # queue head alongside wt/xt so both queues feed the startup; the
        # scalar queue streams the later groups' sel from t=0.
        # per-(group, j) chunks: fine-grained completion feeds the gather
        # pipeline incrementally (coarser chunks stall the PE at boundaries)
        def sel_dma(eng, gi, js=(0, 2, 1)):
            G = GROUPS[gi]
            for j in js:          # match gather consumption order
                b0, cols = SEL_BASE[gi] + j * 3 * G * 128, 3 * G * 128
                eng.dma_start(S[:, b0:b0 + cols], selt_d[:, b0:b0 + cols])

        for gi in range(len(GROUPS)):
            sel_dma(nc.scalar, gi)

        bnds = [0, 810, NT]
        nc.sync.dma_start(wt[:, 0:512], wt_d[:, 0:512])
        for h in range(2):
            nc.sync.dma_start(xt[:, h * NT:h * NT + bnds[1]],
                              xt_d[:, h * NT:h * NT + bnds[1]])
        nc.sync.dma_start(wt[:, 512:], wt_d[:, 512:])
        for h in range(2):
            a, b = h * NT + bnds[1], h * NT + bnds[2]
            nc.sync.dma_start(xt[:, a:b], xt_d[:, a:b])

        warm = wp.tile([128, 128], BF16)
        nc.gpsimd.memset(warm[:], 0)


        for gi, G in enumerate(GROUPS):
            t0 = G_T0[gi]
            gw = G * 128          # moving columns in this group
            # gated patches, per h: xts_h[c, kk*gw + t*128 + p]
            #   = xt[c, h*NT + (t0+t)*128 + i*80 + j + p] * sel[kk, pixel]
            # one op per (h, j): out and sel are contiguous, only the xt
            # gather is strided; even j is 4B-aligned -> DVE 2x mode.
            # gpsimd (mode-agnostic) takes the misaligned j=1 ops.
            xts = [xtsp.tile([128, 9 * gw], BF16, name=f"xts{h}")
                   for h in range(2)]
            for j in (0, 2, 1):
                for h in range(2):
                    # all on vector: DVE tensor_tensor and any gpsimd op
                    # fight for the same shared SBUF port (exclusive lock),
                    # so splitting across engines only adds blocking.
                    # j=1 is 2B-misaligned -> 1x mode; still fits under PE.
                    o = _ap(xts[h], j * 3 * gw, [[1, 3 * gw]])
                    i1 = _ap(xt, h * NT + t0 * 128 + j,
                             [[80, 3], [128, G], [1, 128]])
                    i2 = _ap(S, SEL_BASE[gi] + j * 3 * gw, [[1, 3 * gw]])
                    nc.vector.tensor_mul(o, i1, i2)

            # PE: stationary = w[kk,h,dh] chunk, moving = gated patches.
            # z[d, dh*1024 + p], accumulated over (kk, h) in PSUM.
            z = zp.tile([128, 2048], F32)
            if gi == 0 and WARMUP:
                # warm up the PE's HAM clock gate while input DMAs ramp;
                # overwritten by the first real accumulation (start=True)
                for _ in range(WARMUP):
                    nc.tensor.matmul(z[:, 0:64], warm[:], warm[:, 0:64],
                                     start=True, stop=True,
                                     skip_group_check=True)
            outt = outp.tile([128, 2 * gw], BF16)

            def mm(kk, h, dh, ki):
                wc = wt[:, ((kk * 2 + h) * 2 + dh) * 128:
                        ((kk * 2 + h) * 2 + dh + 1) * 128]
                for n0 in range(0, gw, 512):
                    n1 = min(n0 + 512, gw)
                    nc.tensor.matmul(
                        z[:, dh * 1024 + n0:dh * 1024 + n1],
                        wc,
                        xts[h][:, kk * gw + n0:kk * gw + n1],
                        start=(ki == 0 and h == 0),
                        stop=(ki == 8 and h == 1))

            def evac(dh):
                # PSUM -> SBUF bf16, then contiguous DMA to DRAM [d, p]
                nc.scalar.copy(outt[:, dh * gw:(dh + 1) * gw],
                               z[:, dh * 1024:dh * 1024 + gw])
                nc.sync.dma_start(
                    out_d[dh * 128:(dh + 1) * 128, t0 * 128:t0 * 128 + gw],
                    outt[:, dh * gw:(dh + 1) * gw])

            for ki, kk in enumerate(KK_ORDER):
                for h in range(2):
                    for dh in range(2):
                        mm(kk, h, dh, ki)
            evac(0)
            evac(1)
    nc.compile()
    return nc


_NC_CACHE = None


def _get_nc():
    global _NC_CACHE
    if _NC_CACHE is None:
        _NC_CACHE = _build_nc()
    return _NC_CACHE


def _prep_core(x, seg_mask, core):
    b, r0 = core // 2, 40 * (core % 2)
    xp = np.pad(x[b], ((1, 1), (0, 0), (0, 0)))        # [82,80,256]
    strip = xp[r0:r0 + 42].reshape(42 * W, CH)
    sp = np.zeros((NT, CH), np.float32)
    sp[1:1 + 42 * W] = strip
    spT = sp.T
    xt = np.ascontiguousarray(
        np.concatenate([spT[:128], spT[128:]], axis=1)).astype(BF16NP)

    pads = np.pad(seg_mask[b], ((1, 1), (1, 1), (0, 0)))  # [82,82,22]
    mc = seg_mask[b][r0:r0 + 40]                          # [40,80,22]
    smax = mc.max(-1, keepdims=True)
    eq = (mc == smax).astype(np.float32)
    sel = np.empty((40, 80, 9), np.float32)
    for k in range(9):
        di, dj = k // 3 - 1, k % 3 - 1
        sel[..., k] = (eq * pads[r0 + 1 + di:r0 + 41 + di,
                                 1 + dj:81 + dj]).sum(-1)
    cnt = (sel != 0).astype(np.float32).sum(-1, keepdims=True)
    selp = (sel * (9.0 / np.maximum(cnt, 1.0))).reshape(NPIX, 9)
    # group-major, kk = j*3+i ordered: col = SEL_BASE[g] + kk*(G*128) + t*128 + p
    KK2K = [(kk % 3) * 3 + kk // 3 for kk in range(9)]   # kk -> k = i*3+j
    parts = []
    for gi, G in enumerate(GROUPS):
        t0 = G_T0[gi]
        blk = selp[t0 * 128:(t0 + G) * 128][:, KK2K]     # [G*128, kk]
        parts.append(blk.reshape(G, 128, 9).transpose(2, 0, 1).reshape(-1))
    selt = np.concatenate(parts).astype(BF16NP).reshape(1, NTILE * 9 * 128)
    # pre-broadcast across the 128 partitions (plain contiguous DMA on-chip)
    selt = np.ascontiguousarray(np.broadcast_to(selt, (128, NTILE * 9 * 128)))
    return xt, selt


def _build_in_maps(x, seg_mask, conv_w):
    w9 = conv_w.reshape(2, 128, 9, 2, 128)               # [h, c, k, dh, d]
    KK2K = [(kk % 3) * 3 + kk // 3 for kk in range(9)]
    # wt[c, ((kk*2+h)*2+dh)*128 + d] = w[h*128+c, KK2K[kk], dh*128+d]
    wt = np.ascontiguousarray(
        w9[:, :, KK2K]                                   # [h, c, kk, dh, d]
          .transpose(1, 2, 0, 3, 4)                      # [c, kk, h, dh, d]
          .reshape(128, 36 * 128)).astype(BF16NP)

    in_maps = []
    for core in range(8):
        xt, selt = _prep_core(x, seg_mask, core)
        in_maps.append({"xt": xt, "wt": wt, "selt": selt})
    return in_maps


def kernel(x, seg_mask, conv_w):
    x = np.asarray(x, np.float32)
    seg_mask = np.asarray(seg_mask, np.float32)
    conv_w = np.asarray(conv_w, np.float32)

    in_maps = _build_in_maps(x, seg_mask, conv_w)
    nc = _get_nc()
    res = run_bass_kernel_spmd(nc, in_maps, core_ids=list(range(8)))

    out = np.empty((B, H, W, DIM), np.float32)
    for core in range(8):
        b, r0 = core // 2, 40 * (core % 2)
        o = res.results[core]["out"].astype(np.float32)   # [256, 3200]
        out[b, r0:r0 + 40] = o.T.reshape(ROWS, W, DIM)
    return out
